# revision 38
# baseline (speedup 1.0000x reference)
"""Trainium2 Bass kernel for nn_Decoder (MusicVAE-style hierarchical decoder).

Strategy (8 NeuronCores, data-parallel over batch, no inter-core comms):
  - Conductor LSTM (16 sequential levels, batch 32/core) computes per-level
    embeddings.
  - Decoder levels are INDEPENDENT (initial state from dec_h0/dec_c0,
    note0=0), so all 16 levels are batched: effective decoder batch
    16*32 = 512 rows per core, 16 sequential note steps.
  - The conductor embedding is constant within a level, so its gate
    contribution ge = emb @ dec_Wih[:, :H].T + dec_b is precomputed once.
  - Everything feature-major: [features on partitions, rows free].
  - All matmuls run in fp8 e4m3 with DoubleRow perf mode (2x PE throughput,
    K=256 per instruction).  Weights are pre-scaled by WS=32 on the host to
    avoid the e4m3 denormal zone; the 1/WS dequant is folded into the
    scalar_tensor_tensor gate adds / activation scale (exact powers of 2).
  - PSUM gate layout [128, 4, RC] with slot order (i, f, o, g) so one fused
    DVE scalar_tensor_tensor does psum/WS + ge, and one fused sigmoid covers
    i,f,o.  Elementwise work is balanced across DVE / Scalar / GPSIMD.
  - The decoder's 16 levels split into two independent row-cohorts whose
    steps interleave, hiding each cohort's serial drain/oproj tail under the
    other's matmul phase.  The conductor runs p-chunks in quads; the ge
    half for levels 0-7 is computed inside the conductor's second half.
  - c state in bf16; h/note/emb in fp8 (matmul operands), output in bf16.
"""
import numpy as np
import ml_dtypes

import concourse.bacc as bacc
import concourse.tile as tile
import concourse.mybir as mybir
from concourse.bass_utils import run_bass_kernel_spmd

bf16 = ml_dtypes.bfloat16
f8 = ml_dtypes.float8_e4m3
F32 = mybir.dt.float32
BF = mybir.dt.bfloat16
F8 = mybir.dt.float8e4
AF = mybir.ActivationFunctionType
ALU = mybir.AluOpType
DR = mybir.MatmulPerfMode.DoubleRow

NCORES = 8
B, Z, H, T = 256, 512, 1024, 512
L, NS = 16, 16
Bc = B // NCORES            # 32 batch rows per core
R = L * Bc                  # 512 decoder rows per core (levels x batch)
HK, TK, ZK = H // 128, T // 128, Z // 128   # 8, 4, 4
G = 4 * H // 128            # 32 gate chunks of 128
HP, TP, ZP = HK // 2, TK // 2, ZK // 2      # k-pair counts (fp8 DoubleRow)
RC2 = R // 2                # decoder cohort row-half
WS = 32.0                   # fp8 weight pre-scale
IVS = 1.0 / WS

# gate slot order per p: (i, f, o, g) -> column chunk in 4H
def _gate_ms(p):
    return (p, HK + p, 3 * HK + p, 2 * HK + p)


# slot index for gate-group g (0:i 1:f 2:g 3:o) in the (i,f,o,g) psum layout
_SLOT_OF_GROUP = {0: 0, 1: 1, 2: 3, 3: 2}


def _declare(nc):
    d = {}
    ei = dict(kind="ExternalInput")
    d["ones"] = nc.dram_tensor("ones", [1, R], BF, **ei)
    d["cbias"] = nc.dram_tensor("cbias", [128, G], F32, **ei)
    d["dbias"] = nc.dram_tensor("dbias", [128, G], F32, **ei)
    d["obias"] = nc.dram_tensor("obias", [1, T], BF, **ei)     # 32*out_b
    d["zT"] = nc.dram_tensor("zT", [128, ZK, R], F8, **ei)
    d["h0T"] = nc.dram_tensor("h0T", [128, HK, R], F8, **ei)
    d["c0T"] = nc.dram_tensor("c0T", [128, HK, R], BF, **ei)
    d["cwih"] = nc.dram_tensor("cwih", [128, ZK, 4 * H], F8, **ei)
    d["cwhh"] = nc.dram_tensor("cwhh", [128, HK, 4 * H], F8, **ei)
    d["dwe"] = nc.dram_tensor("dwe", [G, 128, HK * 128], F8, **ei)
    d["dwn"] = nc.dram_tensor("dwn", [128, TK, 4 * H], F8, **ei)
    d["dwhh"] = nc.dram_tensor("dwhh", [128, HK, 4 * H], F8, **ei)
    d["owt"] = nc.dram_tensor("owt", [128, HK, T], F8, **ei)
    d["outbuf"] = nc.dram_tensor("outbuf", [NS, TK, 128, R], BF,
                                 kind="ExternalOutput")
    return d


def _mm8(nc, out, w, x, start, stop):
    return nc.tensor.matmul(out, w, x, start=start, stop=stop, perf_mode=DR)


def _body(nc, tc, d):
    import contextlib
    with contextlib.ExitStack() as ctx:
        Pp = ctx.enter_context(tc.tile_pool(name="persist", bufs=1))

        t_ones = Pp.tile([1, R], BF, tag="ones")
        nc.scalar.dma_start(t_ones[:], d["ones"][:])
        t_ob = Pp.tile([1, T], BF, tag="obias")
        nc.scalar.dma_start(t_ob[:], d["obias"][:])
        t_emb8 = Pp.tile([128, HK, R], F8, tag="emb8")
        t_h = [Pp.tile([128, HK, R], F8, tag=f"hT{i}", name=f"hT{i}")
               for i in (0, 1)]
        t_c = Pp.tile([128, HK, R], BF, tag="c")
        t_note8 = Pp.tile([128, TK, R], F8, tag="note8")
        t_noteb = Pp.tile([128, TK, R], BF, tag="noteb")
        nc.gpsimd.dma_start(t_h[0][:], d["h0T"][:])
        nc.gpsimd.dma_start(t_c[:], d["c0T"][:])

        # ge persists through the decoder; allocated early so the ge halves
        # can be computed inside the conductor window (geA overlaps the
        # second half of the conductor levels).
        Pge = ctx.enter_context(tc.tile_pool(name="gepool", bufs=1))
        t_ge = Pge.tile([128, HK, 4, R], BF, tag="ge")
        # decoder weights pool created early (stack order); DMAs emitted
        # after the conductor so SWDGE overlaps the geB phase
        Pw = ctx.enter_context(tc.tile_pool(name="wdec", bufs=1))
        t_dwn = Pw.tile([128, TK, 4 * H], F8, tag="dwn")
        t_dwhh = Pw.tile([128, HK, 4 * H], F8, tag="dwhh")
        t_owt = Pw.tile([128, HK, T], F8, tag="owt")
        gectx = contextlib.ExitStack()
        Pgw = gectx.enter_context(tc.tile_pool(name="gew", bufs=2))
        PSg = gectx.enter_context(tc.tile_pool(name="geps", bufs=4,
                                               space="PSUM"))
        t_db = Pgw.tile([128, G], F32, tag="dbias", bufs=1)
        nc.scalar.dma_start(t_db[:], d["dbias"][:])

        def ge_block(mb, rs):
            """ge for gate chunks 2*mb..2*mb+1 (one blocked dwe DMA,
            double-buffered so the next block's DMA overlaps compute)."""
            wt = Pgw.tile([128, 2, HK, 128], F8, tag="dwe", bufs=2)
            nc.sync.dma_start(wt[:], d["dwe"][2 * mb:2 * mb + 2].rearrange(
                "m p (k j) -> p m k j", k=HK))
            for mi in range(2):
                m = 2 * mb + mi
                ps = PSg.tile([128, RC2], F32, tag="gep")
                for j in range(HP):
                    _mm8(nc, ps[:], wt[:, mi, 2 * j:2 * j + 2, :],
                         t_emb8[:, 2 * j:2 * j + 2, rs], j == 0, j == HP - 1)
                p_, s_ = m % HK, _SLOT_OF_GROUP[m // HK]
                if m % 2 == 0:
                    nc.vector.tensor_scalar(t_ge[:, p_, s_, rs], ps[:], IVS,
                                            t_db[:, m:m + 1],
                                            op0=ALU.mult, op1=ALU.add)
                else:
                    nc.scalar.activation(t_ge[:, p_, s_, rs], ps[:],
                                         AF.Identity, bias=t_db[:, m:m + 1],
                                         scale=IVS)

        # ---------------- conductor ----------------
        with tc.tile_pool(name="cond", bufs=1) as Pc, \
             tc.tile_pool(name="ctmp", bufs=2) as Pt, \
             tc.tile_pool(name="cps", bufs=2, space="PSUM") as PSc, \
             tc.tile_pool(name="gzps", bufs=2, space="PSUM") as PSz:
            t_cb = Pc.tile([128, G], F32, tag="cbias")
            t_cwhh = Pc.tile([128, HK, 4 * H], F8, tag="cwhh")
            nc.scalar.dma_start(t_cwhh[:], d["cwhh"][:])
            nc.scalar.dma_start(t_dwhh[:], d["dwhh"][:])
            nc.gpsimd.dma_start(t_dwn[:], d["dwn"][:])
            nc.gpsimd.dma_start(t_owt[:], d["owt"][:])
            # gz laid out p-major with slots (i,f,o,g), like the decoder
            t_gz = Pc.tile([128, HK, 4, R], BF, tag="gz")
            t_cc = Pc.tile([128, HK, Bc], F32, tag="cc")

            # gz = z @ cond_Wih.T + cond_b for all levels at once (fp8 DR)
            with tc.tile_pool(name="gzin", bufs=1) as Pcz:
                t_cwih = Pcz.tile([128, ZK, 4 * H], F8, tag="cwih")
                nc.sync.dma_start(t_cwih[:], d["cwih"][:])
                t_zT = Pcz.tile([128, ZK, R], F8, tag="zT")
                nc.sync.dma_start(t_zT[:], d["zT"][:])
                nc.sync.dma_start(t_cb[:], d["cbias"][:])
                for m in range(G):
                    ms = slice(m * 128, (m + 1) * 128)
                    ps = PSz.tile([128, R], F32, tag="gzp")
                    for j in range(ZP):
                        _mm8(nc, ps[:], t_cwih[:, 2 * j:2 * j + 2, ms],
                             t_zT[:, 2 * j:2 * j + 2, :], j == 0, j == ZP - 1)
                    p_, s_ = m % HK, _SLOT_OF_GROUP[m // HK]
                    if m % 2 == 0:
                        nc.vector.tensor_scalar(t_gz[:, p_, s_, :], ps[:], IVS,
                                                t_cb[:, m:m + 1],
                                                op0=ALU.mult, op1=ALU.add)
                    else:
                        nc.scalar.activation(t_gz[:, p_, s_, :], ps[:],
                                             AF.Identity,
                                             bias=t_cb[:, m:m + 1],
                                             scale=IVS)

            # sequential levels, p-chunks processed in quads (ops fused x4
            # to amortize per-op init; muls on the otherwise-idle gpsimd)
            for lv in range(L):
                cs = slice(lv * Bc, (lv + 1) * Bc)
                ps_prev = slice((lv - 1) * Bc, lv * Bc)
                for q in range(2):
                    qs = slice(4 * q, 4 * q + 4)
                    if lv == 0:
                        sio = Pt.tile([128, 4, 3, Bc], BF, tag="csio")
                        tg = Pt.tile([128, 4, Bc], BF, tag="ctg")
                        nc.scalar.activation(sio[:], t_gz[:, qs, 0:3, cs],
                                             AF.Sigmoid)
                        nc.scalar.activation(tg[:], t_gz[:, qs, 3, cs],
                                             AF.Tanh)
                        nc.vector.tensor_mul(t_cc[:, qs, :], sio[:, :, 0, :],
                                             tg[:])
                    else:
                        ps = PSc.tile([128, 4, 4, Bc], F32, tag="cgp")
                        for qi in range(4):
                            gms = _gate_ms(4 * q + qi)
                            for si in range(4):
                                ms = slice(gms[si] * 128,
                                           (gms[si] + 1) * 128)
                                for j in range(HP):
                                    _mm8(nc, ps[:, qi, si, :],
                                         t_cwhh[:, 2 * j:2 * j + 2, ms],
                                         t_emb8[:, 2 * j:2 * j + 2, ps_prev],
                                         j == 0, j == HP - 1)
                        gs = Pt.tile([128, 4, 4, Bc], BF, tag="cgs")
                        nc.vector.scalar_tensor_tensor(
                            gs[:], ps[:], IVS, t_gz[:, qs, :, cs],
                            op0=ALU.mult, op1=ALU.add)
                        sio = Pt.tile([128, 4, 3, Bc], BF, tag="csio")
                        tg = Pt.tile([128, 4, Bc], BF, tag="ctg")
                        nc.scalar.activation(sio[:], gs[:, :, 0:3, :],
                                             AF.Sigmoid)
                        nc.scalar.activation(tg[:], gs[:, :, 3, :], AF.Tanh)
                        tm1 = Pt.tile([128, 4, Bc], BF, tag="ctm1")
                        tm2 = Pt.tile([128, 4, Bc], F32, tag="ctm2")
                        nc.vector.tensor_mul(tm1[:], sio[:, :, 0, :], tg[:])
                        nc.vector.tensor_mul(tm2[:], sio[:, :, 1, :],
                                             t_cc[:, qs, :])
                        nc.vector.tensor_add(t_cc[:, qs, :], tm1[:], tm2[:])
                    tcn = Pt.tile([128, 4, Bc], BF, tag="ctcn")
                    nc.scalar.activation(tcn[:], t_cc[:, qs, :], AF.Tanh)
                    nc.vector.tensor_mul(t_emb8[:, qs, cs], sio[:, :, 2, :],
                                         tcn[:])
                if lv >= L - 8:
                    # geA: rows of levels 0-7 only need emb8[:, :, 0:256],
                    # ready since level 7 — fill the idle engines here
                    ge_block(2 * (lv - (L - 8)), slice(0, RC2))
                    ge_block(2 * (lv - (L - 8)) + 1, slice(0, RC2))


        # geB: rows of levels 8-15
        for mb in range(G // 2):
            ge_block(mb, slice(RC2, R))
        gectx.close()

        # ---------------- decoder: 16 note steps over 512 rows --------------
        # The 16 levels split into two independent row-cohorts (RC=256 rows
        # each) whose steps interleave: cohort A's serial drain/oproj tail
        # hides under cohort B's matmul phase.  Within a cohort-step the
        # emission is software-pipelined (stage A at p, stage B at p-1) so
        # each engine's in-order queue never waits mid-chain.
        RC = R // 2
        with tc.tile_pool(name="dtmp", bufs=3) as Pdt, \
             tc.tile_pool(name="dps", bufs=4, space="PSUM") as PSd:
            stage = {}

            def stage_a(rs, pp, gs):
                # activations for the p-pair (2pp, 2pp+1) fused
                sio = Pdt.tile([128, 2, 3, RC], BF, tag="sio")
                tg = Pdt.tile([128, 2, RC], BF, tag="tg")
                nc.scalar.activation(sio[:], gs[:, :, 0:3, :], AF.Sigmoid)
                nc.scalar.activation(tg[:], gs[:, :, 3, :], AF.Tanh)
                tm1 = Pdt.tile([128, 2, RC], BF, tag="tm1")
                nc.gpsimd.tensor_mul(tm1[:], sio[:, :, 0, :], tg[:])
                stage[pp] = (sio, tm1)

            def stage_b(rs, pp, hout):
                sio, tm1 = stage.pop(pp)
                ps = slice(2 * pp, 2 * pp + 2)
                tm2 = Pdt.tile([128, 2, RC], BF, tag="tm2")
                tcn = Pdt.tile([128, 2, RC], BF, tag="tcn")
                eng2 = nc.gpsimd if pp % 2 == 0 else nc.vector
                eng2.tensor_mul(tm2[:], sio[:, :, 1, :], t_c[:, ps, rs])
                nc.vector.tensor_add(t_c[:, ps, rs], tm1[:], tm2[:])
                nc.scalar.activation(tcn[:], t_c[:, ps, rs], AF.Tanh)
                nc.gpsimd.tensor_mul(hout[:, ps, rs], sio[:, :, 2, :], tcn[:])

            def cohort_step(t, rh):
                rs = slice(rh * RC, (rh + 1) * RC)
                hin = t_h[t % 2]
                hout = t_h[(t + 1) % 2]
                for pp in range(HK // 2):
                    gs = Pdt.tile([128, 2, 4, RC], BF, tag="gs")
                    for pi in range(2):
                        p = 2 * pp + pi
                        gms = _gate_ms(p)
                        pt = PSd.tile([128, 4, RC], F32, tag="dgp")
                        for si in range(4):
                            ms = slice(gms[si] * 128, (gms[si] + 1) * 128)
                            if t > 0:
                                # note contribution first: note8(t-1) is the
                                # freshest input, h pairs follow
                                for j in range(TP):
                                    _mm8(nc, pt[:, si, :],
                                         t_dwn[:, 2 * j:2 * j + 2, ms],
                                         t_note8[:, 2 * j:2 * j + 2, rs],
                                         j == 0, False)
                            for j in range(HP):
                                _mm8(nc, pt[:, si, :],
                                     t_dwhh[:, 2 * j:2 * j + 2, ms],
                                     hin[:, 2 * j:2 * j + 2, rs],
                                     (j == 0 and t == 0), j == HP - 1)
                        nc.vector.scalar_tensor_tensor(
                            gs[:, pi, :, :], pt[:], IVS, t_ge[:, p, :, rs],
                            op0=ALU.mult, op1=ALU.add)
                    stage_a(rs, pp, gs)
                    if pp >= 1:
                        stage_b(rs, pp - 1, hout)
                stage_b(rs, HK // 2 - 1, hout)
                # output projection + sigmoid -> bf16 out + fp8 feedback copy
                po = PSd.tile([128, TK, RC], F32, tag="dgp", name="po")
                for tk in range(TK):
                    ts_ = slice(tk * 128, (tk + 1) * 128)
                    nc.tensor.matmul(po[:, tk, :], t_ob[0:1, ts_],
                                     t_ones[0:1, rs], start=True, stop=False)
                    for j in range(HP):
                        _mm8(nc, po[:, tk, :], t_owt[:, 2 * j:2 * j + 2, ts_],
                             hout[:, 2 * j:2 * j + 2, rs], False, j == HP - 1)
                nc.scalar.activation(t_noteb[:, :, rs], po[:], AF.Sigmoid,
                                     scale=IVS)
                if t < NS - 1:
                    eng8 = nc.vector if rh == 0 else nc.gpsimd
                    eng8.tensor_copy(t_note8[:, :, rs], t_noteb[:, :, rs])
                for tk in range(TK):
                    nc.sync.dma_start(d["outbuf"][t, tk, :, rs],
                                      t_noteb[:, tk, rs])

            for t in range(NS):
                cohort_step(t, 0)
                cohort_step(t, 1)


_CACHE = {}


def _build():
    if "nc" not in _CACHE:
        nc = bacc.Bacc("TRN2", target_bir_lowering=False, debug=False,
                       num_devices=NCORES)
        d = _declare(nc)
        with tile.TileContext(nc) as tc:
            _body(nc, tc, d)
        nc.compile()
        _CACHE["nc"] = nc
    return _CACHE["nc"]


def _q8(x):
    return np.clip(x, -240.0, 240.0).astype(f8)


def _feat_major(W):
    """[J, K] -> [128, K/128, J] (stationary lhsT chunk layout)."""
    J, K = W.shape
    return np.ascontiguousarray(
        W.reshape(J, K // 128, 128).transpose(2, 1, 0))


def _pack_inputs(inputs):
    z = np.asarray(inputs["z"], np.float32)
    dec_h0 = np.asarray(inputs["dec_h0"], np.float32)
    dec_c0 = np.asarray(inputs["dec_c0"], np.float32)
    cond_b = np.asarray(inputs["cond_bih"] + inputs["cond_bhh"], np.float32)
    dec_b = np.asarray(inputs["dec_bih"] + inputs["dec_bhh"], np.float32)
    out_b = np.asarray(inputs["out_b"], np.float32)

    shared = {
        "ones": np.ones((1, R), dtype=bf16),
        "cbias": np.ascontiguousarray(cond_b.reshape(G, 128).T).astype(np.float32),
        "dbias": np.ascontiguousarray(dec_b.reshape(G, 128).T).astype(np.float32),
        "obias": (WS * out_b)[None, :].astype(bf16),
        "cwih": _q8(WS * _feat_major(np.asarray(inputs["cond_Wih"], np.float32))),
        "cwhh": _q8(WS * _feat_major(np.asarray(inputs["cond_Whh"], np.float32))),
        "dwn": _q8(WS * _feat_major(np.asarray(inputs["dec_Wih"][:, H:], np.float32))),
        "dwhh": _q8(WS * _feat_major(np.asarray(inputs["dec_Whh"], np.float32))),
        "owt": _q8(WS * _feat_major(np.asarray(inputs["out_W"], np.float32))),
    }
    dwe_fm = _q8(WS * _feat_major(np.asarray(inputs["dec_Wih"][:, :H], np.float32)))
    # slab m: [128, HK*128] so each DMA is one contiguous read
    shared["dwe"] = np.ascontiguousarray(
        dwe_fm.reshape(128, HK, G, 128).transpose(2, 0, 1, 3).reshape(
            G, 128, HK * 128))

    z_lv = z[:, np.arange(L) * L, 0, :]           # [B, L, Z]
    in_maps = []
    for c in range(NCORES):
        bs = slice(c * Bc, (c + 1) * Bc)
        zc = z_lv[bs]                              # [Bc, L, Z]
        zT = _q8(np.ascontiguousarray(
            zc.reshape(Bc, L, ZK, 128).transpose(3, 2, 1, 0).reshape(128, ZK, R)))
        h0 = dec_h0[:, bs, :]                      # [L, Bc, H]
        h0T = np.ascontiguousarray(
            h0.reshape(L, Bc, HK, 128).transpose(3, 2, 0, 1).reshape(128, HK, R))
        c0 = dec_c0[:, bs, :]
        c0T = np.ascontiguousarray(
            c0.reshape(L, Bc, HK, 128).transpose(3, 2, 0, 1).reshape(128, HK, R))
        m = dict(shared)
        m["zT"] = zT
        m["h0T"] = _q8(h0T)
        m["c0T"] = c0T.astype(bf16)
        in_maps.append(m)
    return in_maps


def _unpack_outputs(core_outs):
    notes = np.empty((B, L * NS, T), np.float32)
    for c, arr in enumerate(core_outs):
        # arr [NS, TK, 128, R] -> [Bc, L, NS, T]
        a = arr.astype(np.float32).reshape(NS, TK, 128, L, Bc).transpose(4, 3, 0, 1, 2)
        notes[c * Bc:(c + 1) * Bc] = a.reshape(Bc, L, NS, T).reshape(
            Bc, L * NS, T)
    return notes


def kernel(**inputs):
    nc = _build()
    in_maps = _pack_inputs(inputs)
    res = run_bass_kernel_spmd(nc, in_maps, list(range(NCORES)))
    return _unpack_outputs([r["outbuf"] for r in res.results])


# revision 39
# speedup vs baseline: 1.0079x; 1.0079x over previous
"""Trainium2 Bass kernel for nn_Decoder (MusicVAE-style hierarchical decoder).

Strategy (8 NeuronCores, data-parallel over batch, no inter-core comms):
  - Conductor LSTM (16 sequential levels, batch 32/core) computes per-level
    embeddings.
  - Decoder levels are INDEPENDENT (initial state from dec_h0/dec_c0,
    note0=0), so all 16 levels are batched: effective decoder batch
    16*32 = 512 rows per core, 16 sequential note steps.
  - The conductor embedding is constant within a level, so its gate
    contribution ge = emb @ dec_Wih[:, :H].T + dec_b is precomputed once.
  - Everything feature-major: [features on partitions, rows free].
  - All matmuls run in fp8 e4m3 with DoubleRow perf mode (2x PE throughput,
    K=256 per instruction).  Weights are pre-scaled by WS=32 on the host to
    avoid the e4m3 denormal zone; the 1/WS dequant is folded into the
    scalar_tensor_tensor gate adds / activation scale (exact powers of 2).
  - PSUM gate layout [128, 4, RC] with slot order (i, f, o, g) so one fused
    DVE scalar_tensor_tensor does psum/WS + ge, and one fused sigmoid covers
    i,f,o.  Elementwise work is balanced across DVE / Scalar / GPSIMD.
  - The decoder's 16 levels split into two independent row-cohorts whose
    steps interleave, hiding each cohort's serial drain/oproj tail under the
    other's matmul phase.  The conductor runs p-chunks in quads; the ge
    half for levels 0-7 is computed inside the conductor's second half.
  - c state in bf16; h/note/emb in fp8 (matmul operands), output in bf16.
"""
import numpy as np
import ml_dtypes

import concourse.bacc as bacc
import concourse.tile as tile
import concourse.mybir as mybir
from concourse.bass_utils import run_bass_kernel_spmd

bf16 = ml_dtypes.bfloat16
f8 = ml_dtypes.float8_e4m3
F32 = mybir.dt.float32
BF = mybir.dt.bfloat16
F8 = mybir.dt.float8e4
AF = mybir.ActivationFunctionType
ALU = mybir.AluOpType
DR = mybir.MatmulPerfMode.DoubleRow

NCORES = 8
B, Z, H, T = 256, 512, 1024, 512
L, NS = 16, 16
Bc = B // NCORES            # 32 batch rows per core
R = L * Bc                  # 512 decoder rows per core (levels x batch)
HK, TK, ZK = H // 128, T // 128, Z // 128   # 8, 4, 4
G = 4 * H // 128            # 32 gate chunks of 128
HP, TP, ZP = HK // 2, TK // 2, ZK // 2      # k-pair counts (fp8 DoubleRow)
RC2 = R // 2                # decoder cohort row-half
WS = 32.0                   # fp8 weight pre-scale
IVS = 1.0 / WS

# gate slot order per p: (i, f, o, g) -> column chunk in 4H
def _gate_ms(p):
    return (p, HK + p, 3 * HK + p, 2 * HK + p)


# slot index for gate-group g (0:i 1:f 2:g 3:o) in the (i,f,o,g) psum layout
_SLOT_OF_GROUP = {0: 0, 1: 1, 2: 3, 3: 2}


def _declare(nc):
    d = {}
    ei = dict(kind="ExternalInput")
    d["ones"] = nc.dram_tensor("ones", [1, R], BF, **ei)
    d["cbias"] = nc.dram_tensor("cbias", [128, G], F32, **ei)
    d["dbias"] = nc.dram_tensor("dbias", [128, G], F32, **ei)
    d["obias"] = nc.dram_tensor("obias", [1, T], BF, **ei)     # 32*out_b
    d["zT"] = nc.dram_tensor("zT", [128, ZK, R], F8, **ei)
    d["h0T"] = nc.dram_tensor("h0T", [128, HK, R], F8, **ei)
    d["c0T"] = nc.dram_tensor("c0T", [128, HK, R], BF, **ei)
    d["cwih"] = nc.dram_tensor("cwih", [128, ZK, 4 * H], F8, **ei)
    d["cwhh"] = nc.dram_tensor("cwhh", [128, HK, 4 * H], F8, **ei)
    d["dwe"] = nc.dram_tensor("dwe", [G, 128, HK * 128], F8, **ei)
    d["dwn"] = nc.dram_tensor("dwn", [128, TK, 4 * H], F8, **ei)
    d["dwhh"] = nc.dram_tensor("dwhh", [128, HK, 4 * H], F8, **ei)
    d["owt"] = nc.dram_tensor("owt", [128, HK, T], F8, **ei)
    d["outbuf"] = nc.dram_tensor("outbuf", [NS, TK, 128, R], BF,
                                 kind="ExternalOutput")
    return d


def _mm8(nc, out, w, x, start, stop):
    return nc.tensor.matmul(out, w, x, start=start, stop=stop, perf_mode=DR)


def _body(nc, tc, d):
    import contextlib
    with contextlib.ExitStack() as ctx:
        Pp = ctx.enter_context(tc.tile_pool(name="persist", bufs=1))

        t_ones = Pp.tile([1, R], BF, tag="ones")
        nc.scalar.dma_start(t_ones[:], d["ones"][:])
        t_ob = Pp.tile([1, T], BF, tag="obias")
        nc.scalar.dma_start(t_ob[:], d["obias"][:])
        t_emb8 = Pp.tile([128, HK, R], F8, tag="emb8")
        t_h = [Pp.tile([128, HK, R], F8, tag=f"hT{i}", name=f"hT{i}")
               for i in (0, 1)]
        t_c = Pp.tile([128, HK, R], BF, tag="c")
        t_note8 = Pp.tile([128, TK, R], F8, tag="note8")
        t_noteb = Pp.tile([128, TK, R], BF, tag="noteb")
        nc.gpsimd.dma_start(t_h[0][:], d["h0T"][:])
        nc.gpsimd.dma_start(t_c[:], d["c0T"][:])

        # ge persists through the decoder; allocated early so the ge halves
        # can be computed inside the conductor window (geA overlaps the
        # second half of the conductor levels).
        Pge = ctx.enter_context(tc.tile_pool(name="gepool", bufs=1))
        t_ge = Pge.tile([128, HK, 4, R], BF, tag="ge")
        # decoder weights pool created early (stack order); DMAs emitted
        # after the conductor so SWDGE overlaps the geB phase
        Pw = ctx.enter_context(tc.tile_pool(name="wdec", bufs=1))
        t_dwn = Pw.tile([128, TK, 4 * H], F8, tag="dwn")
        t_dwhh = Pw.tile([128, HK, 4 * H], F8, tag="dwhh")
        t_owt = Pw.tile([128, HK, T], F8, tag="owt")
        gectx = contextlib.ExitStack()
        Pgw = gectx.enter_context(tc.tile_pool(name="gew", bufs=2))
        PSg = gectx.enter_context(tc.tile_pool(name="geps", bufs=3,
                                               space="PSUM"))
        t_db = Pgw.tile([128, G], F32, tag="dbias", bufs=1)
        nc.scalar.dma_start(t_db[:], d["dbias"][:])

        def ge_block(mb, rs):
            """ge for gate chunks 2*mb..2*mb+1 (one blocked dwe DMA,
            double-buffered so the next block's DMA overlaps compute)."""
            wt = Pgw.tile([128, 2, HK, 128], F8, tag="dwe", bufs=2)
            nc.sync.dma_start(wt[:], d["dwe"][2 * mb:2 * mb + 2].rearrange(
                "m p (k j) -> p m k j", k=HK))
            for mi in range(2):
                m = 2 * mb + mi
                ps = PSg.tile([128, RC2], F32, tag="gep")
                for j in range(HP):
                    _mm8(nc, ps[:], wt[:, mi, 2 * j:2 * j + 2, :],
                         t_emb8[:, 2 * j:2 * j + 2, rs], j == 0, j == HP - 1)
                p_, s_ = m % HK, _SLOT_OF_GROUP[m // HK]
                if m % 2 == 0:
                    nc.vector.tensor_scalar(t_ge[:, p_, s_, rs], ps[:], IVS,
                                            t_db[:, m:m + 1],
                                            op0=ALU.mult, op1=ALU.add)
                else:
                    nc.scalar.activation(t_ge[:, p_, s_, rs], ps[:],
                                         AF.Identity, bias=t_db[:, m:m + 1],
                                         scale=IVS)

        # ---------------- conductor ----------------
        with tc.tile_pool(name="cond", bufs=1) as Pc, \
             tc.tile_pool(name="ctmp", bufs=2) as Pt, \
             tc.tile_pool(name="cps", bufs=2, space="PSUM") as PSc, \
             tc.tile_pool(name="gzps", bufs=3, space="PSUM") as PSz:
            t_cb = Pc.tile([128, G], F32, tag="cbias")
            t_cwhh = Pc.tile([128, HK, 4 * H], F8, tag="cwhh")
            nc.scalar.dma_start(t_cwhh[:], d["cwhh"][:])
            nc.scalar.dma_start(t_dwhh[:], d["dwhh"][:])
            nc.gpsimd.dma_start(t_dwn[:], d["dwn"][:])
            nc.gpsimd.dma_start(t_owt[:], d["owt"][:])
            # gz laid out p-major with slots (i,f,o,g), like the decoder
            t_gz = Pc.tile([128, HK, 4, R], BF, tag="gz")
            t_cc = Pc.tile([128, HK, Bc], F32, tag="cc")

            # gz = z @ cond_Wih.T + cond_b for all levels at once (fp8 DR)
            with tc.tile_pool(name="gzin", bufs=1) as Pcz:
                t_cwih = Pcz.tile([128, ZK, 4 * H], F8, tag="cwih")
                nc.sync.dma_start(t_cwih[:], d["cwih"][:])
                t_zT = Pcz.tile([128, ZK, R], F8, tag="zT")
                nc.sync.dma_start(t_zT[:], d["zT"][:])
                nc.sync.dma_start(t_cb[:], d["cbias"][:])
                for m in range(G):
                    ms = slice(m * 128, (m + 1) * 128)
                    ps = PSz.tile([128, R], F32, tag="gzp")
                    for j in range(ZP):
                        _mm8(nc, ps[:], t_cwih[:, 2 * j:2 * j + 2, ms],
                             t_zT[:, 2 * j:2 * j + 2, :], j == 0, j == ZP - 1)
                    p_, s_ = m % HK, _SLOT_OF_GROUP[m // HK]
                    if m % 2 == 0:
                        nc.vector.tensor_scalar(t_gz[:, p_, s_, :], ps[:], IVS,
                                                t_cb[:, m:m + 1],
                                                op0=ALU.mult, op1=ALU.add)
                    else:
                        nc.scalar.activation(t_gz[:, p_, s_, :], ps[:],
                                             AF.Identity,
                                             bias=t_cb[:, m:m + 1],
                                             scale=IVS)

            # sequential levels, p-chunks processed in quads (ops fused x4
            # to amortize per-op init; muls on the otherwise-idle gpsimd)
            for lv in range(L):
                cs = slice(lv * Bc, (lv + 1) * Bc)
                ps_prev = slice((lv - 1) * Bc, lv * Bc)
                for q in range(2):
                    qs = slice(4 * q, 4 * q + 4)
                    if lv == 0:
                        sio = Pt.tile([128, 4, 3, Bc], BF, tag="csio")
                        tg = Pt.tile([128, 4, Bc], BF, tag="ctg")
                        nc.scalar.activation(sio[:], t_gz[:, qs, 0:3, cs],
                                             AF.Sigmoid)
                        nc.scalar.activation(tg[:], t_gz[:, qs, 3, cs],
                                             AF.Tanh)
                        nc.vector.tensor_mul(t_cc[:, qs, :], sio[:, :, 0, :],
                                             tg[:])
                    else:
                        ps = PSc.tile([128, 4, 4, Bc], F32, tag="cgp")
                        for qi in range(4):
                            gms = _gate_ms(4 * q + qi)
                            for si in range(4):
                                ms = slice(gms[si] * 128,
                                           (gms[si] + 1) * 128)
                                for j in range(HP):
                                    _mm8(nc, ps[:, qi, si, :],
                                         t_cwhh[:, 2 * j:2 * j + 2, ms],
                                         t_emb8[:, 2 * j:2 * j + 2, ps_prev],
                                         j == 0, j == HP - 1)
                        gs = Pt.tile([128, 4, 4, Bc], BF, tag="cgs")
                        nc.vector.scalar_tensor_tensor(
                            gs[:], ps[:], IVS, t_gz[:, qs, :, cs],
                            op0=ALU.mult, op1=ALU.add)
                        sio = Pt.tile([128, 4, 3, Bc], BF, tag="csio")
                        tg = Pt.tile([128, 4, Bc], BF, tag="ctg")
                        nc.scalar.activation(sio[:], gs[:, :, 0:3, :],
                                             AF.Sigmoid)
                        nc.scalar.activation(tg[:], gs[:, :, 3, :], AF.Tanh)
                        tm1 = Pt.tile([128, 4, Bc], BF, tag="ctm1")
                        tm2 = Pt.tile([128, 4, Bc], F32, tag="ctm2")
                        nc.vector.tensor_mul(tm1[:], sio[:, :, 0, :], tg[:])
                        nc.vector.tensor_mul(tm2[:], sio[:, :, 1, :],
                                             t_cc[:, qs, :])
                        nc.vector.tensor_add(t_cc[:, qs, :], tm1[:], tm2[:])
                    tcn = Pt.tile([128, 4, Bc], BF, tag="ctcn")
                    nc.scalar.activation(tcn[:], t_cc[:, qs, :], AF.Tanh)
                    nc.vector.tensor_mul(t_emb8[:, qs, cs], sio[:, :, 2, :],
                                         tcn[:])
                if lv >= L - 8:
                    # geA: rows of levels 0-7 only need emb8[:, :, 0:256],
                    # ready since level 7 — fill the idle engines here
                    ge_block(2 * (lv - (L - 8)), slice(0, RC2))
                    ge_block(2 * (lv - (L - 8)) + 1, slice(0, RC2))


        # geB: rows of levels 8-15
        for mb in range(G // 2):
            ge_block(mb, slice(RC2, R))
        gectx.close()

        # ---------------- decoder: 16 note steps over 512 rows --------------
        # The 16 levels split into two independent row-cohorts (RC=256 rows
        # each) whose steps interleave: cohort A's serial drain/oproj tail
        # hides under cohort B's matmul phase.  Within a cohort-step the
        # emission is software-pipelined (stage A at p, stage B at p-1) so
        # each engine's in-order queue never waits mid-chain.
        RC = R // 2
        with tc.tile_pool(name="dtmp", bufs=3) as Pdt, \
             tc.tile_pool(name="dps", bufs=4, space="PSUM") as PSd:
            stage = {}

            def stage_a(rs, pp, gs):
                # activations for the p-pair (2pp, 2pp+1) fused
                sio = Pdt.tile([128, 2, 3, RC], BF, tag="sio")
                tg = Pdt.tile([128, 2, RC], BF, tag="tg")
                nc.scalar.activation(sio[:], gs[:, :, 0:3, :], AF.Sigmoid)
                nc.scalar.activation(tg[:], gs[:, :, 3, :], AF.Tanh)
                tm1 = Pdt.tile([128, 2, RC], BF, tag="tm1")
                nc.gpsimd.tensor_mul(tm1[:], sio[:, :, 0, :], tg[:])
                stage[pp] = (sio, tm1)

            def stage_b(rs, pp, hout):
                sio, tm1 = stage.pop(pp)
                ps = slice(2 * pp, 2 * pp + 2)
                tm2 = Pdt.tile([128, 2, RC], BF, tag="tm2")
                tcn = Pdt.tile([128, 2, RC], BF, tag="tcn")
                eng2 = nc.gpsimd if pp % 2 == 0 else nc.vector
                eng2.tensor_mul(tm2[:], sio[:, :, 1, :], t_c[:, ps, rs])
                nc.vector.tensor_add(t_c[:, ps, rs], tm1[:], tm2[:])
                nc.scalar.activation(tcn[:], t_c[:, ps, rs], AF.Tanh)
                nc.gpsimd.tensor_mul(hout[:, ps, rs], sio[:, :, 2, :], tcn[:])

            def cohort_step(t, rh):
                rs = slice(rh * RC, (rh + 1) * RC)
                hin = t_h[t % 2]
                hout = t_h[(t + 1) % 2]
                for pp in range(HK // 2):
                    gs = Pdt.tile([128, 2, 4, RC], BF, tag="gs")
                    for pi in range(2):
                        p = 2 * pp + pi
                        gms = _gate_ms(p)
                        pt = PSd.tile([128, 4, RC], F32, tag="dgp")
                        for si in range(4):
                            ms = slice(gms[si] * 128, (gms[si] + 1) * 128)
                            if t > 0:
                                # note contribution first: note8(t-1) is the
                                # freshest input, h pairs follow
                                for j in range(TP):
                                    _mm8(nc, pt[:, si, :],
                                         t_dwn[:, 2 * j:2 * j + 2, ms],
                                         t_note8[:, 2 * j:2 * j + 2, rs],
                                         j == 0, False)
                            for j in range(HP):
                                _mm8(nc, pt[:, si, :],
                                     t_dwhh[:, 2 * j:2 * j + 2, ms],
                                     hin[:, 2 * j:2 * j + 2, rs],
                                     (j == 0 and t == 0), j == HP - 1)
                        nc.vector.scalar_tensor_tensor(
                            gs[:, pi, :, :], pt[:], IVS, t_ge[:, p, :, rs],
                            op0=ALU.mult, op1=ALU.add)
                    stage_a(rs, pp, gs)
                    if pp >= 1:
                        stage_b(rs, pp - 1, hout)
                stage_b(rs, HK // 2 - 1, hout)
                # output projection + sigmoid -> bf16 out + fp8 feedback copy
                po = PSd.tile([128, TK, RC], F32, tag="dgp", name="po")
                for tk in range(TK):
                    ts_ = slice(tk * 128, (tk + 1) * 128)
                    nc.tensor.matmul(po[:, tk, :], t_ob[0:1, ts_],
                                     t_ones[0:1, rs], start=True, stop=False)
                    for j in range(HP):
                        _mm8(nc, po[:, tk, :], t_owt[:, 2 * j:2 * j + 2, ts_],
                             hout[:, 2 * j:2 * j + 2, rs], False, j == HP - 1)
                nc.scalar.activation(t_noteb[:, :, rs], po[:], AF.Sigmoid,
                                     scale=IVS)
                if t < NS - 1:
                    eng8 = nc.vector if rh == 0 else nc.gpsimd
                    eng8.tensor_copy(t_note8[:, :, rs], t_noteb[:, :, rs])
                for tk in range(TK):
                    nc.sync.dma_start(d["outbuf"][t, tk, :, rs],
                                      t_noteb[:, tk, rs])

            for t in range(NS):
                cohort_step(t, 0)
                cohort_step(t, 1)


_CACHE = {}


def _build():
    if "nc" not in _CACHE:
        nc = bacc.Bacc("TRN2", target_bir_lowering=False, debug=False,
                       num_devices=NCORES)
        d = _declare(nc)
        with tile.TileContext(nc) as tc:
            _body(nc, tc, d)
        nc.compile()
        _CACHE["nc"] = nc
    return _CACHE["nc"]


def _q8(x):
    return np.clip(x, -240.0, 240.0).astype(f8)


def _feat_major(W):
    """[J, K] -> [128, K/128, J] (stationary lhsT chunk layout)."""
    J, K = W.shape
    return np.ascontiguousarray(
        W.reshape(J, K // 128, 128).transpose(2, 1, 0))


def _pack_inputs(inputs):
    z = np.asarray(inputs["z"], np.float32)
    dec_h0 = np.asarray(inputs["dec_h0"], np.float32)
    dec_c0 = np.asarray(inputs["dec_c0"], np.float32)
    cond_b = np.asarray(inputs["cond_bih"] + inputs["cond_bhh"], np.float32)
    dec_b = np.asarray(inputs["dec_bih"] + inputs["dec_bhh"], np.float32)
    out_b = np.asarray(inputs["out_b"], np.float32)

    shared = {
        "ones": np.ones((1, R), dtype=bf16),
        "cbias": np.ascontiguousarray(cond_b.reshape(G, 128).T).astype(np.float32),
        "dbias": np.ascontiguousarray(dec_b.reshape(G, 128).T).astype(np.float32),
        "obias": (WS * out_b)[None, :].astype(bf16),
        "cwih": _q8(WS * _feat_major(np.asarray(inputs["cond_Wih"], np.float32))),
        "cwhh": _q8(WS * _feat_major(np.asarray(inputs["cond_Whh"], np.float32))),
        "dwn": _q8(WS * _feat_major(np.asarray(inputs["dec_Wih"][:, H:], np.float32))),
        "dwhh": _q8(WS * _feat_major(np.asarray(inputs["dec_Whh"], np.float32))),
        "owt": _q8(WS * _feat_major(np.asarray(inputs["out_W"], np.float32))),
    }
    dwe_fm = _q8(WS * _feat_major(np.asarray(inputs["dec_Wih"][:, :H], np.float32)))
    # slab m: [128, HK*128] so each DMA is one contiguous read
    shared["dwe"] = np.ascontiguousarray(
        dwe_fm.reshape(128, HK, G, 128).transpose(2, 0, 1, 3).reshape(
            G, 128, HK * 128))

    z_lv = z[:, np.arange(L) * L, 0, :]           # [B, L, Z]
    in_maps = []
    for c in range(NCORES):
        bs = slice(c * Bc, (c + 1) * Bc)
        zc = z_lv[bs]                              # [Bc, L, Z]
        zT = _q8(np.ascontiguousarray(
            zc.reshape(Bc, L, ZK, 128).transpose(3, 2, 1, 0).reshape(128, ZK, R)))
        h0 = dec_h0[:, bs, :]                      # [L, Bc, H]
        h0T = np.ascontiguousarray(
            h0.reshape(L, Bc, HK, 128).transpose(3, 2, 0, 1).reshape(128, HK, R))
        c0 = dec_c0[:, bs, :]
        c0T = np.ascontiguousarray(
            c0.reshape(L, Bc, HK, 128).transpose(3, 2, 0, 1).reshape(128, HK, R))
        m = dict(shared)
        m["zT"] = zT
        m["h0T"] = _q8(h0T)
        m["c0T"] = c0T.astype(bf16)
        in_maps.append(m)
    return in_maps


def _unpack_outputs(core_outs):
    notes = np.empty((B, L * NS, T), np.float32)
    for c, arr in enumerate(core_outs):
        # arr [NS, TK, 128, R] -> [Bc, L, NS, T]
        a = arr.astype(np.float32).reshape(NS, TK, 128, L, Bc).transpose(4, 3, 0, 1, 2)
        notes[c * Bc:(c + 1) * Bc] = a.reshape(Bc, L, NS, T).reshape(
            Bc, L * NS, T)
    return notes


def kernel(**inputs):
    nc = _build()
    in_maps = _pack_inputs(inputs)
    res = run_bass_kernel_spmd(nc, in_maps, list(range(NCORES)))
    return _unpack_outputs([r["outbuf"] for r in res.results])
